# revision 18
# baseline (speedup 1.0000x reference)
"""Trainium2 Bass kernel: batched tiny-window attention (B=6272, N=8, C=768, H=12).

Data-parallel over 8 NeuronCores (784 batches / 6272 tokens per core).
Per-core fused pipeline, fp16 compute, fp32 accumulate:
  x -> (PE transpose) xT[c,tok] -> qkv matmul -> qT/kT [oc,tok] + v [tok,oc]
    -> per-128-token-group block-diag attention (S=qT.T@kT, multiplicative
       exp(bias) mask, softmax) -> AT -> out_h = v_h.T @ AT -> attnT[c,tok]
    -> proj matmul -> out [tok, C] -> DMA.
Scale (hd^-0.5) and qkv bias are folded into host-side precomputed weights.
"""

import os
import sys
from contextlib import ExitStack

import numpy as np

sys.path.insert(0, "/opt/trn_rl_repo")

import concourse.bass as bass  # noqa: E402
import concourse.bacc as bacc  # noqa: E402
import concourse.tile as tile  # noqa: E402
from concourse import mybir  # noqa: E402
from concourse.bass_utils import run_bass_kernel_spmd  # noqa: E402
from concourse.masks import make_identity  # noqa: E402

NCORES = 8
B, N, C = 6272, 8, 768
H, HD = 12, 64
OC = 3 * C
B_LOC = B // NCORES          # 784 batches per core
TOK = B_LOC * N              # 6272 tokens per core
CCH = C // 128               # 6 channel chunks
GRP = 128                    # tokens per attention group (16 batches)
MACRO = 512                  # tokens per macro tile

F16 = mybir.dt.float16
F32 = mybir.dt.float32

LAST_RESULT = {}             # test harness introspection (exec_time_ns etc.)


def _build_nc(use_bias: bool):
    nc = bacc.Bacc()
    x_ext = nc.declare_dram_parameter("x", [TOK, C], F32, isOutput=False)
    wqkv_ext = nc.declare_dram_parameter("wqkvT", [C, OC], F16, isOutput=False)
    wproj_ext = nc.declare_dram_parameter("wprojT", [C, C], F16, isOutput=False)
    bm_ext = nc.declare_dram_parameter("bmask", [H, GRP, GRP], F32, isOutput=False)
    if use_bias:
        qkb_ext = nc.declare_dram_parameter("qkb", [2 * C], F32, isOutput=False)
        vb_ext = nc.declare_dram_parameter("vb", [C], F32, isOutput=False)
    out_ext = nc.declare_dram_parameter("out", [TOK, C], F32, isOutput=True)

    macros = []
    t0 = 0
    while t0 < TOK:
        tw = min(MACRO, TOK - t0)
        macros.append((t0, tw))
        t0 += tw

    with tile.TileContext(nc) as tc, ExitStack() as ctx:
        wpool = ctx.enter_context(tc.tile_pool(name="weights", bufs=1))
        xf32p = ctx.enter_context(tc.tile_pool(name="xf32", bufs=8))
        xTp = ctx.enter_context(tc.tile_pool(name="xT", bufs=12))
        qkTp = ctx.enter_context(tc.tile_pool(name="qkT", bufs=18))
        vp = ctx.enter_context(tc.tile_pool(name="v", bufs=6))
        attp = ctx.enter_context(tc.tile_pool(name="attnT", bufs=12))
        smallp = ctx.enter_context(tc.tile_pool(name="small", bufs=10))
        statp = ctx.enter_context(tc.tile_pool(name="stat", bufs=12))
        outp = ctx.enter_context(tc.tile_pool(name="outsb", bufs=3))
        ps_small = ctx.enter_context(tc.tile_pool(name="ps_s", bufs=4, space="PSUM"))
        ps_big = ctx.enter_context(tc.tile_pool(name="ps_b", bufs=2, space="PSUM"))

        # --- persistent weights / masks / identities ---
        id_f32 = wpool.tile([128, 128], F32)
        make_identity(nc, id_f32)
        id_f16 = wpool.tile([128, 128], F16)
        make_identity(nc, id_f16)

        wqkv = []
        for c in range(CCH):
            wt = wpool.tile([128, OC], F16, tag=f"wqkv{c}", name="wt")
            nc.sync.dma_start(out=wt, in_=wqkv_ext.ap()[c * 128:(c + 1) * 128, :])
            wqkv.append(wt)
        wproj = []
        for c in range(CCH):
            wt = wpool.tile([128, C], F16, tag=f"wproj{c}", name="wt")
            nc.sync.dma_start(out=wt, in_=wproj_ext.ap()[c * 128:(c + 1) * 128, :])
            wproj.append(wt)
        bmask = []
        for h in range(H):
            bt = wpool.tile([128, 128], F32, tag=f"bmask{h}", name="bt")
            nc.sync.dma_start(out=bt, in_=bm_ext.ap()[h])
            bmask.append(bt)

        qkb_t = vb_t = None
        if use_bias:
            qkb_t = wpool.tile([128, 2 * CCH], F32)
            nc.sync.dma_start(
                out=qkb_t, in_=qkb_ext.ap().rearrange("(a p) -> p a", p=128))
            vb_t = wpool.tile([128, C], F32)
            nc.sync.dma_start(out=vb_t, in_=vb_ext.ap().to_broadcast((128, C)))

        # --- main loop over macro tiles ---
        for (t0, tw) in macros:
            nsub = tw // GRP

            # Phase A: load x and transpose to xT[c] = [128c, tw] f16.
            # All sub-tile transposes of one c-chunk land in a single psum
            # bank so each xT[c] has exactly one writer (walrus wait limit).
            xT = [xTp.tile([128, MACRO], F16, tag="xt", name="xt") for _ in range(CCH)]
            xin = [xf32p.tile([128, C], F32, tag="xin", name="xin")
                   for _ in range(nsub)]
            for s in range(nsub):
                nc.sync.dma_start(
                    out=xin[s], in_=x_ext.ap()[t0 + s * GRP: t0 + (s + 1) * GRP, :])
            for c in range(CCH):
                pst = ps_big.tile([128, 1024], F32, tag="big")
                for s in range(nsub):
                    nc.tensor.transpose(
                        out=pst[:, s * GRP:(s + 1) * GRP],
                        in_=xin[s][:, c * 128:(c + 1) * 128], identity=id_f32)
                nc.scalar.copy(out=xT[c][:, :tw], in_=pst[:, :tw])

            # Phase B: qkv.  q/k in [oc, tok] layout; v in [tok, oc] layout.
            qkT = [qkTp.tile([128, MACRO], F16, tag="qkt", name="qkt") for _ in range(2 * CCH)]
            for j in range(2 * CCH):
                psq = ps_big.tile([128, 1024], F32, tag="big", name="psq")
                for c in range(CCH):
                    nc.tensor.matmul(
                        psq[:, :tw],
                        lhsT=wqkv[c][:, j * 128:(j + 1) * 128],
                        rhs=xT[c][:, :tw],
                        start=(c == 0), stop=(c == CCH - 1))
                if use_bias:
                    nc.vector.tensor_scalar(
                        out=qkT[j][:, :tw], in0=psq[:, :tw],
                        scalar1=qkb_t[:, j:j + 1], scalar2=None,
                        op0=mybir.AluOpType.add)
                else:
                    nc.vector.tensor_copy(out=qkT[j][:, :tw], in_=psq[:, :tw])

            vt = [vp.tile([128, C], F16, tag="vt", name="vt") for _ in range(nsub)]
            for s in range(nsub):
                psv = ps_big.tile([128, 1024], F32, tag="big", name="psv")
                for c in range(CCH):
                    nc.tensor.matmul(
                        psv[:, 0:384],
                        lhsT=xT[c][:, s * GRP:(s + 1) * GRP],
                        rhs=wqkv[c][:, 2 * C:2 * C + 384],
                        start=(c == 0), stop=(c == CCH - 1))
                    nc.tensor.matmul(
                        psv[:, 512:896],
                        lhsT=xT[c][:, s * GRP:(s + 1) * GRP],
                        rhs=wqkv[c][:, 2 * C + 384:OC],
                        start=(c == 0), stop=(c == CCH - 1))
                if use_bias:
                    nc.vector.tensor_tensor(
                        out=vt[s].rearrange("p (a f) -> p a f", f=384),
                        in0=psv.rearrange("p (a f) -> p a f", f=512)[:, :, 0:384],
                        in1=vb_t.rearrange("p (a f) -> p a f", f=384),
                        op=mybir.AluOpType.add)
                else:
                    nc.vector.tensor_copy(
                        out=vt[s].rearrange("p (a f) -> p a f", f=384),
                        in_=psv.rearrange("p (a f) -> p a f", f=512)[:, :, 0:384])

            # Phase C: attention per 128-token group, per head.
            attnT = [attp.tile([128, MACRO], F16, tag="att", name="att") for _ in range(CCH)]
            for s in range(nsub):
                gsl = slice(s * GRP, (s + 1) * GRP)
                for j in range(CCH):            # head pair (2j, 2j+1)
                    at_sb = []
                    for half in range(2):
                        h = 2 * j + half
                        psl = slice(64 * half, 64 * half + 64)
                        sps = ps_small.tile([128, 128], F32, tag="pss")
                        nc.tensor.matmul(
                            sps, lhsT=qkT[j][psl, gsl],
                            rhs=qkT[CCH + j][psl, gsl], start=True, stop=True)
                        # A = exp(S) * exp_bias_mask ; rowsum via fused reduce
                        ex = smallp.tile([128, 128], F32, tag="ex")
                        nc.scalar.activation(
                            out=ex, in_=sps,
                            func=mybir.ActivationFunctionType.Exp)
                        a_t = smallp.tile([128, 128], F16, tag="a")
                        rs = statp.tile([128, 1], F32, tag="rs")
                        nc.vector.tensor_tensor(
                            out=a_t, in0=ex, in1=bmask[h],
                            op=mybir.AluOpType.mult)
                        nc.vector.tensor_reduce(
                            out=rs, in_=a_t, axis=mybir.AxisListType.X,
                            op=mybir.AluOpType.add)
                        rc = statp.tile([128, 1], F32, tag="rc")
                        nc.vector.reciprocal(out=rc, in_=rs)
                        a_n = smallp.tile([128, 128], F16, tag="an")
                        nc.vector.tensor_scalar(
                            out=a_n, in0=a_t, scalar1=rc[:, 0:1], scalar2=None,
                            op0=mybir.AluOpType.mult)
                        atp = ps_small.tile([128, 128], F16, tag="pss")
                        nc.tensor.transpose(out=atp, in_=a_n, identity=id_f16)
                        at1 = smallp.tile([128, 128], F16, tag="at", name="at1")
                        nc.scalar.copy(out=at1, in_=atp)
                        at_sb.append(at1)
                    # Both heads' outputs into one psum tile (partition halves)
                    ops = ps_small.tile([128, 128], F32, tag="pss")
                    nc.tensor.matmul(
                        ops[0:64, :], lhsT=vt[s][:, 2 * j * 64:(2 * j + 1) * 64],
                        rhs=at_sb[0], start=True, stop=True)
                    nc.tensor.matmul(
                        ops[64:128, :],
                        lhsT=vt[s][:, (2 * j + 1) * 64:(2 * j + 2) * 64],
                        rhs=at_sb[1], start=True, stop=True,
                        tile_position=(0, 64))
                    nc.scalar.copy(out=attnT[j][:, gsl], in_=ops)

            # Phase D: proj back to [tok, C], DMA out.
            for s in range(nsub):
                psp = ps_big.tile([128, 1024], F32, tag="big", name="psp")
                for c in range(CCH):
                    nc.tensor.matmul(
                        psp[:, 0:384],
                        lhsT=attnT[c][:, s * GRP:(s + 1) * GRP],
                        rhs=wproj[c][:, 0:384],
                        start=(c == 0), stop=(c == CCH - 1))
                    nc.tensor.matmul(
                        psp[:, 512:896],
                        lhsT=attnT[c][:, s * GRP:(s + 1) * GRP],
                        rhs=wproj[c][:, 384:768],
                        start=(c == 0), stop=(c == CCH - 1))
                osb = outp.tile([128, C], F32, tag="osb")
                nc.scalar.copy(
                    out=osb.rearrange("p (a f) -> p a f", f=384),
                    in_=psp.rearrange("p (a f) -> p a f", f=512)[:, :, 0:384])
                nc.sync.dma_start(
                    out=out_ext.ap()[t0 + s * GRP: t0 + (s + 1) * GRP, :], in_=osb)

    nc.compile()
    return nc


_NC_CACHE = None


def kernel(x, qkv_w, qkv_b, proj_w, proj_b, rel_bias_table):
    global _NC_CACHE
    x = np.asarray(x, np.float32)
    qkv_w = np.asarray(qkv_w, np.float32)
    qkv_b = np.asarray(qkv_b, np.float32)
    proj_w = np.asarray(proj_w, np.float32)
    proj_b = np.asarray(proj_b, np.float32)
    tbl = np.asarray(rel_bias_table, np.float32)

    scale = HD ** -0.5
    # Fold attention scale into the q block of the qkv weight (and bias).
    wq = qkv_w.copy()
    wq[:C] *= scale
    bq = qkv_b.copy()
    bq[:C] *= scale
    wqkvT = np.ascontiguousarray(wq.T).astype(np.float16)          # [C, 3C]
    wprojT = np.ascontiguousarray(proj_w.T).astype(np.float16)     # [C, C]

    # Block-diagonal multiplicative mask: exp(rel bias) inside each 8-token
    # batch block, 0 elsewhere.  bm[h, t, t'] for a 128-token group.
    bm = np.zeros((H, GRP, GRP), np.float32)
    eb = np.exp(tbl)                                               # [15, H]
    for b in range(GRP // N):
        for n in range(N):
            for m in range(N):
                bm[:, b * N + n, b * N + m] = eb[m - n + N - 1, :]

    use_bias = bool(np.any(qkv_b != 0))
    xs = x.reshape(NCORES, TOK, C)
    in_maps = []
    for i in range(NCORES):
        m = {"x": np.ascontiguousarray(xs[i]), "wqkvT": wqkvT,
             "wprojT": wprojT, "bmask": bm}
        if use_bias:
            m["qkb"] = np.ascontiguousarray(bq[:2 * C])
            m["vb"] = np.ascontiguousarray(qkv_b[2 * C:])
        in_maps.append(m)

    if _NC_CACHE is None or _NC_CACHE[0] != use_bias:
        _NC_CACHE = (use_bias, _build_nc(use_bias))
    nc = _NC_CACHE[1]

    trace = bool(int(os.environ.get("KERNEL_TRACE", "0")))
    res = run_bass_kernel_spmd(nc, in_maps, core_ids=list(range(NCORES)),
                               trace=trace)
    LAST_RESULT["exec_time_ns"] = getattr(res, "exec_time_ns", None)
    out = np.concatenate([np.asarray(r["out"]) for r in res.results], axis=0)
    out = out.reshape(B, N, C).astype(np.float32)
    if np.any(proj_b != 0):
        out = out + proj_b[None, None, :]
    return out
